# revision 1
# baseline (speedup 1.0000x reference)
"""Fused additive-attention kernel for Trainium2 (8 NeuronCores, SPMD).

Computes  w = softmax_K( mask ? (Wl . tanh(vW_v^T + qW_q^T) + bl) : -1e9 )
without ever materializing the [B,N,S,K,H] joint_repr intermediate.

Sharding: data-parallel over batch B (16) across 8 cores -> 2 batches/core.
Weights replicated. Host does layout prep only (transposes / packing); all
FLOPs (matmuls, tanh, softmax) run on device.

Per-core dataflow (h on partitions for the hot loop), phased per h-chunk so
compute starts before all weights arrive:
  qpT[hc] [128(h), 512(b,ns)] = WqT-slice.T @ qT        (PE, psum acc, bf16)
  vpT     duplicated-pair table VP2[p, 2k]=VP2[p, 2k+1] (so the broadcast add
          below can use an inner step-1 pair AP -> DVE 2x_1P mode)
  JT      [128, (kk,b,ns)] bf16 = qpT + vp[b,k]         (DVE tensor_tensor,
          vp read via [[0,128],[1,2]] broadcast AP)
  tanh in-place on JT (one big ACT op per (hc, k-group))
  logits  psum [57, 512]: rows 0:25 = k<25, rows 32:57 = k>=25, accumulated
          with zero-padded Wl lhsT; (k, k+25) share lhsT and run as adjacent
          matmuls on PSUM col-strips 0/32 (tile_position col-tiling).
  masked softmax over k after PE-transposing logits to [ns, k].
"""

import os
import sys

import numpy as np

sys.path.insert(0, "/opt/trn_rl_repo")

import concourse.bass as bass
import concourse.mybir as mybir
from concourse import bacc, bass_utils
from concourse.tile import TileContext

# Problem shapes (hardcoded per contract -- kernel.py must be self-contained)
B, N, S, K = 16, 4, 64, 50
VD, QD, H = 1024, 768, 512
NCORES = 8
BPC = B // NCORES          # batches per core = 2
NS = BPC * N * S           # 512 rows (b, n, s) per core
KB = BPC * K               # 100 (b, k) columns per core
HC = H // 128              # 4 h-chunks
QC = QD // 128             # 6 qd-chunks
VC = VD // 128             # 8 vd-chunks

# j-groups per h-chunk: lists of Wl-variant indices j (k = j and k = j + 25
# are processed together, sharing the lhsT).  hc0 ramps up with a small first
# group so the first tanh issues as early as possible.
GROUPS_HC0 = [list(range(0, 5)), list(range(5, 15)), list(range(15, 25))]
GROUPS_HCX = [list(range(0, 10)), list(range(10, 20)), list(range(20, 25))]

F32 = mybir.dt.float32
BF16 = mybir.dt.bfloat16

_CACHE = {}


def _build_nc():
    nc = bacc.Bacc("TRN2", target_bir_lowering=False)

    qT_h = nc.dram_tensor("qT", [QD, NS], BF16, kind="ExternalInput")
    vT_h = nc.dram_tensor("vT", [VD, KB], BF16, kind="ExternalInput")
    # weight slabs, pre-split by hc-pair: A = h cols 0:256, B = 256:512
    WqTA_h = nc.dram_tensor("WqTA", [QD, 256], BF16, kind="ExternalInput")
    WqTB_h = nc.dram_tensor("WqTB", [QD, 256], BF16, kind="ExternalInput")
    WvTA_h = nc.dram_tensor("WvTA", [VD, 256], BF16, kind="ExternalInput")
    WvTB_h = nc.dram_tensor("WvTB", [VD, 256], BF16, kind="ExternalInput")
    # packed [128, 12]: cols 0:4 Wl chunks, 4:8 bq chunks, 8:12 bv chunks
    wlb_h = nc.dram_tensor("wlb", [128, 12], F32, kind="ExternalInput")
    # zero-padded Wl variants: [128, hc*625 + j*25 + c] = Wl[hc*128+p]*(c==j)
    wlz_h = nc.dram_tensor("wlz", [128, HC * 25 * 25], BF16, kind="ExternalInput")
    # packed [128, 200]: cols 0:100 maskf (b,k) replicated, 100:200 (maskf-1)*1e9
    msk_h = nc.dram_tensor("msk", [128, 2 * KB], F32, kind="ExternalInput")
    id_h = nc.dram_tensor("ident", [128, 128], F32, kind="ExternalInput")
    out_h = nc.dram_tensor("out", [NS, K], F32, kind="ExternalOutput")

    with TileContext(nc) as tc:
        with (
            tc.tile_pool(name="persist", bufs=1) as pp,
            tc.tile_pool(name="ppsum", bufs=1, space="PSUM") as ppsum,
            tc.tile_pool(name="smpsum", bufs=2, space="PSUM") as sps,
        ):
            # ---- DMA loads, chunked + ordered so the hc0 projection chain
            # starts after the first (qts, wqtA) chunk instead of all loads ----
            vts = pp.tile([128, VC, KB], BF16, name="vts")
            nc.sync.dma_start(
                vts[:, :, :], vT_h[:, :].rearrange("(c p) j -> p c j", p=128)
            )
            qts = pp.tile([128, QC, NS], BF16, name="qts")
            wqtA = pp.tile([128, QC, 256], BF16, name="wqtA")
            qT_r = qT_h[:, :].rearrange("(c p) j -> p c j", p=128)
            wqA_r = WqTA_h[:, :].rearrange("(c p) j -> p c j", p=128)
            for c in range(3):
                nc.sync.dma_start(
                    qts[:, 2 * c : 2 * c + 2, :], qT_r[:, 2 * c : 2 * c + 2, :]
                )
                nc.sync.dma_start(
                    wqtA[:, 2 * c : 2 * c + 2, :], wqA_r[:, 2 * c : 2 * c + 2, :]
                )
                if c == 0:
                    wvtA = pp.tile([128, VC, 256], BF16, name="wvtA")
                    nc.sync.dma_start(
                        wvtA[:, :, :],
                        WvTA_h[:, :].rearrange("(c p) j -> p c j", p=128),
                    )
            wlb = pp.tile_from(wlb_h[:, :], name="wlb")
            wlz = pp.tile_from(wlz_h[:, :], name="wlz")
            msk = pp.tile_from(msk_h[:, :], name="msk")
            ident = pp.tile_from(id_h[:, :], name="ident")
            wqtB = pp.tile([128, QC, 256], BF16, name="wqtB")
            nc.sync.dma_start(
                wqtB[:, :, :], WqTB_h[:, :].rearrange("(c p) j -> p c j", p=128)
            )
            wvtB = pp.tile([128, VC, 256], BF16, name="wvtB")
            nc.sync.dma_start(
                wvtB[:, :, :], WvTB_h[:, :].rearrange("(c p) j -> p c j", p=128)
            )

            # qpT (all h-chunks): [128, (hc, b, ns)] bf16
            QPs = pp.tile([128, HC * NS], BF16, name="QPs")
            # duplicated-pair vp table: [128, (hc, b, k, 2)] bf16
            VP2 = pp.tile([128, HC * KB * 2], BF16, name="VP2")

            # logits psum [57, 512]: rows 0:25 <- k 0:25 (col strip 0),
            # rows 32:57 <- k 25:50 (col strip 32)
            ps_log = ppsum.tile([57, NS], F32, name="ps_log")

            def proj_phase(ph, wqt, wvt):
                """Compute QPs/VP2 h-chunks [2*ph, 2*ph+2) from slab wqt/wvt."""
                with tc.tile_pool(name=f"p1ps{ph}", bufs=2, space="PSUM") as p1ps:
                    for i in range(2):
                        hc = 2 * ph + i
                        pq = p1ps.tile([128, NS], F32, tag="pq", name="pq")
                        for qc in range(QC):
                            nc.tensor.matmul(
                                pq[:, :],
                                wqt[:, qc, i * 128 : (i + 1) * 128],
                                qts[:, qc, :],
                                start=(qc == 0),
                                stop=(qc == QC - 1),
                            )
                        pv = p1ps.tile([128, KB], F32, tag="pv", name="pv")
                        for vc in range(VC):
                            nc.tensor.matmul(
                                pv[:, :],
                                wvt[:, vc, i * 128 : (i + 1) * 128],
                                vts[:, vc, :],
                                start=(vc == 0),
                                stop=(vc == VC - 1),
                            )
                        nc.vector.tensor_scalar_add(
                            QPs[:, hc * NS : (hc + 1) * NS],
                            pq[:, :],
                            wlb[:, HC + hc : HC + hc + 1],
                        )
                        vp2v = VP2[
                            :, hc * 2 * KB : (hc + 1) * 2 * KB
                        ].rearrange("p (k two) -> p k two", two=2)
                        pv3 = pv[:, :].rearrange("p (k one) -> p k one", one=1)
                        for par in range(2):
                            nc.vector.tensor_scalar_add(
                                vp2v[:, :, par : par + 1],
                                pv3[:, :, :],
                                wlb[:, 2 * HC + hc : 2 * HC + hc + 1],
                            )

            def main_hc(hc, mp, mid_cb=None):
                """Joint tanh + logit matmuls for one h-chunk."""
                groups = GROUPS_HC0 if hc == 0 else GROUPS_HCX
                for g, js in enumerate(groups):
                    if g == 1 and mid_cb is not None:
                        mid_cb()
                    L = len(js)
                    JT = mp.tile([128, 2 * L * NS], BF16, tag="JT", name="JT")
                    for kk in range(2 * L):
                        k = js[kk] if kk < L else js[kk - L] + 25
                        for b in range(BPC):
                            off = kk * NS + b * (NS // BPC)
                            c2 = hc * 2 * KB + (b * K + k) * 2
                            nc.vector.tensor_add(
                                JT[:, off : off + NS // BPC].rearrange(
                                    "p (x c) -> p x c", c=2
                                ),
                                QPs[
                                    :,
                                    hc * NS
                                    + b * (NS // BPC) : hc * NS
                                    + (b + 1) * (NS // BPC),
                                ].rearrange("p (x c) -> p x c", c=2),
                                VP2[:, c2 : c2 + 2]
                                .rearrange("p (x c) -> p x c", x=1)
                                .broadcast_to((128, NS // BPC // 2, 2)),
                            )
                    # in-place tanh over the whole group
                    nc.scalar.activation(
                        JT[:, :], JT[:, :], mybir.ActivationFunctionType.Tanh
                    )
                    for jj in range(L):
                        j = js[jj]
                        first = hc == 0 and g == 0 and jj == 0
                        last = hc == HC - 1 and g == len(groups) - 1 and jj == L - 1
                        nc.tensor.matmul(
                            ps_log[0:25, :],
                            wlz[:, hc * 625 + j * 25 : hc * 625 + (j + 1) * 25],
                            JT[:, jj * NS : (jj + 1) * NS],
                            start=first,
                            stop=last,
                            tile_position=(0, 0),
                            skip_group_check=True,
                        )
                        nc.tensor.matmul(
                            ps_log[32:57, :],
                            wlz[:, hc * 625 + j * 25 : hc * 625 + (j + 1) * 25],
                            JT[:, (L + jj) * NS : (L + jj + 1) * NS],
                            start=first,
                            stop=last,
                            tile_position=(0, 32),
                            skip_group_check=True,
                        )

            def proj_b():
                with tc.high_priority():
                    proj_phase(1, wqtB, wvtB)

            proj_phase(0, wqtA, wvtA)
            with tc.tile_pool(name="main", bufs=3) as mp:
                main_hc(0, mp, mid_cb=proj_b)
                main_hc(1, mp)
                main_hc(2, mp)
                main_hc(3, mp)

            # ---- masked softmax over k ----
            LG0 = pp.tile([25, NS], F32, name="LG0")
            LG1 = pp.tile([57, NS], F32, name="LG1")
            W_all = pp.tile([128, NS // 128, K], F32, name="W_all")
            nc.vector.tensor_copy(LG0[:, :], ps_log[0:25, :])
            nc.vector.tensor_copy(LG1[32:57, :], ps_log[32:57, :])
            for nsc in range(NS // 128):
                b = nsc // ((NS // BPC) // 128)
                LT = pp.tile([128, K], F32, name=f"LT{nsc}")
                for half in range(2):
                    ps_t = sps.tile([128, 25], F32, tag="ps_t", name="ps_t")
                    if half == 0:
                        src = LG0[0:25, nsc * 128 : (nsc + 1) * 128]
                        idn = ident[0:25, 0:25]
                    else:
                        src = LG1[32:57, nsc * 128 : (nsc + 1) * 128]
                        idn = ident[32:57, 32:57]
                    nc.tensor.transpose(ps_t[:, :], src, idn)
                    nc.vector.tensor_copy(
                        LT[:, half * 25 : (half + 1) * 25], ps_t[:, :]
                    )
                # masked = logits*maskf + (maskf-1)*1e9
                nc.vector.tensor_mul(
                    LT[:, :], LT[:, :], msk[:, b * K : (b + 1) * K]
                )
                nc.vector.tensor_add(
                    LT[:, :], LT[:, :], msk[:, KB + b * K : KB + (b + 1) * K]
                )
                mx = pp.tile([128, 1], F32, name=f"mx{nsc}")
                nc.vector.tensor_reduce(
                    mx[:, :], LT[:, :], axis=mybir.AxisListType.X,
                    op=mybir.AluOpType.max,
                )
                mxn = pp.tile([128, 1], F32, name=f"mxn{nsc}")
                nc.vector.tensor_scalar_mul(mxn[:, :], mx[:, :], -1.0)
                EX = pp.tile([128, K], F32, name=f"EX{nsc}")
                sm = pp.tile([128, 1], F32, name=f"sm{nsc}")
                nc.scalar.activation(
                    EX[:, :], LT[:, :], mybir.ActivationFunctionType.Exp,
                    bias=mxn[:, 0:1], accum_out=sm[:, 0:1],
                )
                rs = pp.tile([128, 1], F32, name=f"rs{nsc}")
                nc.vector.reciprocal(rs[:, :], sm[:, :])
                nc.vector.tensor_scalar_mul(
                    W_all[:, nsc, :], EX[:, :], rs[:, 0:1]
                )
            nc.sync.dma_start(
                out_h[:, :].rearrange("(c p) j -> p c j", p=128), W_all[:, :, :]
            )

    nc.finalize()
    return nc


def _prep_in_maps(v, q, box_mask, Wv, bv, Wq, bq, Wl):
    """Host-side layout prep: shard over B, transpose to device layouts."""
    import ml_dtypes

    v = np.asarray(v, np.float32).reshape(B, K, VD)
    q = np.asarray(q, np.float32).reshape(B, N * S, QD)
    mask = np.asarray(box_mask).astype(np.float32).reshape(B, K)

    WqT = np.asarray(Wq, np.float32).T                                # [QD, H]
    WvT = np.asarray(Wv, np.float32).T                                # [VD, H]
    WqTA = np.ascontiguousarray(WqT[:, :256]).astype(ml_dtypes.bfloat16)
    WqTB = np.ascontiguousarray(WqT[:, 256:]).astype(ml_dtypes.bfloat16)
    WvTA = np.ascontiguousarray(WvT[:, :256]).astype(ml_dtypes.bfloat16)
    WvTB = np.ascontiguousarray(WvT[:, 256:]).astype(ml_dtypes.bfloat16)
    wlb = np.zeros((128, 12), np.float32)
    wl_chunks = np.asarray(Wl, np.float32).reshape(4, 128).T          # [128, hc]
    wlb[:, 0:4] = wl_chunks
    wlb[:, 4:8] = np.asarray(bq, np.float32).reshape(4, 128).T
    wlb[:, 8:12] = np.asarray(bv, np.float32).reshape(4, 128).T
    # zero-padded Wl variants: wlz[p, hc*625 + j*25 + c] = Wl_chunk[p,hc]*(c==j)
    wlz = np.zeros((128, HC, 25, 25), np.float32)
    for j in range(25):
        wlz[:, :, j, j] = wl_chunks
    wlz = wlz.reshape(128, HC * 625).astype(ml_dtypes.bfloat16)
    ident = np.eye(128, dtype=np.float32)

    in_maps = []
    for c in range(NCORES):
        b0 = c * BPC
        qc = q[b0 : b0 + BPC].reshape(NS, QD)
        vc = v[b0 : b0 + BPC].reshape(KB, VD)
        qT = np.ascontiguousarray(qc.T).astype(ml_dtypes.bfloat16)    # [QD, NS]
        vT = np.ascontiguousarray(vc.T).astype(ml_dtypes.bfloat16)    # [VD, KB]
        mf = mask[b0 : b0 + BPC].reshape(1, KB)
        msk = np.zeros((128, 2 * KB), np.float32)
        msk[:, :KB] = mf
        msk[:, KB:] = (mf - 1.0) * 1e9
        in_maps.append(
            {
                "qT": qT,
                "vT": vT,
                "WqTA": WqTA,
                "WqTB": WqTB,
                "WvTA": WvTA,
                "WvTB": WvTB,
                "wlb": wlb,
                "wlz": wlz,
                "msk": msk,
                "ident": ident,
            }
        )
    return in_maps


def kernel(v, q, box_mask, tags_attention, Wv, bv, Wq, bq, Wl, bl):
    # bl shifts all unmasked logits uniformly -> cancels in softmax.
    # tags_attention is unused by the reference module.
    if "nc" not in _CACHE:
        _CACHE["nc"] = _build_nc()
    nc = _CACHE["nc"]
    in_maps = _prep_in_maps(v, q, box_mask, Wv, bv, Wq, bq, Wl)
    res = bass_utils.run_bass_kernel_spmd(
        nc,
        in_maps,
        core_ids=list(range(NCORES)),
        trace=bool(os.environ.get("KERNEL_TRACE")),
        tmpdir=os.environ.get("KERNEL_TMPDIR"),
    )
    _CACHE["last_result"] = res
    outs = [r["out"].reshape(BPC, N, S, K) for r in res.results]
    return np.concatenate(outs, axis=0)



# revision 6
# speedup vs baseline: 1.4890x; 1.4890x over previous
"""Fused additive-attention kernel for Trainium2 (8 NeuronCores, SPMD).

Computes  w = softmax_K( mask ? (Wl . tanh(vW_v^T + qW_q^T) + bl) : -1e9 )
without materializing the [B,N,S,K,H] joint_repr intermediate.

Key ideas over the naive formulation:
  * Masked boxes get weight exactly 0 (exp(-1e9) underflows), so only the
    unmasked boxes are computed.  Host gathers each batch's unmasked box
    list; batches are paired onto cores large-with-small so the padded
    per-slot counts (K0 for the core's first batch, K1 for its second) stay
    near the true max.  Masked/padding slots are -1e9'd on device and the
    host scatters results back into the full [B,N,S,K] output (zeros for
    masked boxes).
  * The broadcast add vp[b,k,h] + qp[b,n,s,h] runs as DVE tensor_scalar_add
    with a per-partition [128,1] vp operand -> 4x perf mode (vs 2x for the
    tensor_tensor pair-broadcast trick).
  * Biases bq/bv are folded into QPs/VP at projection time; bl cancels in
    softmax.  Logits are bounded (|logit| <= sum|Wl|·1), so softmax skips
    the max-subtraction pass.

Per-core dataflow (h on partitions for the hot loop), phased per h-chunk so
compute starts before all weights arrive:
  QPs[hc] [128(h), 512(b,ns)] = WqT-slice.T @ qT + bq   (PE psum, DVE copy)
  VP[hc]  [128(h), S(slots)]  = WvT-slice.T @ vG + bv   (S = K0+K1)
  JT      [128, strip(kk)*256] bf16 = QPs-half + VP[slot]  (DVE 4x)
  tanh in-place on JT (one ACT op per slot-group)
  logits  psum [48, 512]: per batch b, slot rows 0:P at col-strip 0 and
          P:2P at col-strip 32 (P = Kb/2), accumulated over hc with
          zero-padded Wl lhsT (pair j, j+P shares lhsT; tile_position).
  softmax over slots after PE-transposing logits to [ns, slots].
"""

import os
import sys

import numpy as np

sys.path.insert(0, "/opt/trn_rl_repo")

import concourse.bass as bass
import concourse.mybir as mybir
from concourse import bacc, bass_utils
from concourse.tile import TileContext

# Problem shapes (hardcoded per contract -- kernel.py must be self-contained)
B, N, S, K = 16, 4, 64, 50
VD, QD, H = 1024, 768, 512
NCORES = 8
BPC = B // NCORES          # batches per core = 2
NS = BPC * N * S           # 512 rows (b, n, s) per core
NSB = NS // BPC            # 256 rows per batch
HC = H // 128              # 4 h-chunks
QC = QD // 128             # 6 qd-chunks
VC = VD // 128             # 8 vd-chunks

F32 = mybir.dt.float32
BF16 = mybir.dt.bfloat16

_CACHE = {}


def _groups(hc, b, P):
    """Pair-index groups for (hc, batch).  First group of the whole kernel is
    tiny so the first tanh issues as early as possible."""
    pairs = list(range(P))
    if hc == 0 and b == 0:
        return [pairs[0:2], pairs[2 : P // 2 + 1], pairs[P // 2 + 1 :]]
    h = (P + 1) // 2
    return [pairs[:h], pairs[h:]]


def _build_nc(K0, K1):
    P0, P1 = K0 // 2, K1 // 2
    SL = K0 + K1               # slots per core
    KMAX = max(K0, K1)
    WZ0 = P0 * P0              # wlz cols per hc for batch 0
    WZC = WZ0 + P1 * P1        # wlz cols per hc total

    nc = bacc.Bacc("TRN2", target_bir_lowering=False)

    qT_h = nc.dram_tensor("qT", [QD, NS], BF16, kind="ExternalInput")
    vG_h = nc.dram_tensor("vG", [VD, SL], BF16, kind="ExternalInput")
    # weight slabs, pre-split by hc-pair: A = h cols 0:256, B = 256:512
    WqTA_h = nc.dram_tensor("WqTA", [QD, 256], BF16, kind="ExternalInput")
    WqTB_h = nc.dram_tensor("WqTB", [QD, 256], BF16, kind="ExternalInput")
    WvTA_h = nc.dram_tensor("WvTA", [VD, 256], BF16, kind="ExternalInput")
    WvTB_h = nc.dram_tensor("WvTB", [VD, 256], BF16, kind="ExternalInput")
    # packed [128, 12]: cols 0:4 Wl chunks, 4:8 bq chunks, 8:12 bv chunks
    wlb_h = nc.dram_tensor("wlb", [128, 12], F32, kind="ExternalInput")
    # zero-padded Wl variants, per (hc, b, j): [128, Pb] slab, col c = Wl*(c==j)
    wlz_h = nc.dram_tensor("wlz", [128, HC * WZC], BF16, kind="ExternalInput")
    # additive mask: col s = 0.0 for a real slot, -1e9 for padding/masked
    msk_h = nc.dram_tensor("msk", [128, SL], F32, kind="ExternalInput")
    id_h = nc.dram_tensor("ident", [128, 128], F32, kind="ExternalInput")
    out_h = nc.dram_tensor("out", [NS, KMAX], F32, kind="ExternalOutput")

    with TileContext(nc) as tc:
        with (
            tc.tile_pool(name="persist", bufs=1) as pp,
            tc.tile_pool(name="ppsum", bufs=1, space="PSUM") as ppsum,
            tc.tile_pool(name="smpsum", bufs=2, space="PSUM") as sps,
        ):
            # ---- DMA loads, chunked + ordered so the hc0 projection chain
            # starts after the first (qts, wqtA) chunk instead of all loads ----
            vts = pp.tile([128, VC, SL], BF16, name="vts")
            nc.sync.dma_start(
                vts[:, :, :], vG_h[:, :].rearrange("(c p) j -> p c j", p=128)
            )
            qts = pp.tile([128, QC, NS], BF16, name="qts")
            wqtA = pp.tile([128, QC, 256], BF16, name="wqtA")
            qT_r = qT_h[:, :].rearrange("(c p) j -> p c j", p=128)
            wqA_r = WqTA_h[:, :].rearrange("(c p) j -> p c j", p=128)
            for c in range(3):
                nc.sync.dma_start(
                    qts[:, 2 * c : 2 * c + 2, :], qT_r[:, 2 * c : 2 * c + 2, :]
                )
                nc.sync.dma_start(
                    wqtA[:, 2 * c : 2 * c + 2, :], wqA_r[:, 2 * c : 2 * c + 2, :]
                )
                if c == 0:
                    wvtA = pp.tile([128, VC, 256], BF16, name="wvtA")
                    nc.sync.dma_start(
                        wvtA[:, :, :],
                        WvTA_h[:, :].rearrange("(c p) j -> p c j", p=128),
                    )
            wlb = pp.tile_from(wlb_h[:, :], name="wlb")
            wlz = pp.tile_from(wlz_h[:, :], name="wlz")
            msk = pp.tile_from(msk_h[:, :], name="msk")
            ident = pp.tile_from(id_h[:, :], name="ident")
            wqtB = pp.tile([128, QC, 256], BF16, name="wqtB")
            nc.sync.dma_start(
                wqtB[:, :, :], WqTB_h[:, :].rearrange("(c p) j -> p c j", p=128)
            )
            wvtB = pp.tile([128, VC, 256], BF16, name="wvtB")
            nc.sync.dma_start(
                wvtB[:, :, :], WvTB_h[:, :].rearrange("(c p) j -> p c j", p=128)
            )

            # qp (all h-chunks): [128, (hc, b, ns)] bf16, +bq folded
            QPs = pp.tile([128, HC * NS], BF16, name="QPs")
            # vp slot table: [128, (hc, slot)] f32, +bv folded
            VP = pp.tile([128, HC * SL], F32, name="VP")

            # logits psum: batch b owns cols b*256:(b+1)*256 and PE col-strips
            # (0, 32) for b0 / (64, 96) for b1 -> psum rows 0:P0, 32:32+P0,
            # 64:64+P1, 96:96+P1.  Strips of the two batches must not share
            # psum partition rows: start=True zeroes the whole 2KB bank row.
            ps_log = ppsum.tile([96 + 32, NS], F32, name="ps_log")

            def proj_phase(ph, wqt, wvt):
                """Compute QPs/VP h-chunks [2*ph, 2*ph+2) from slab wqt/wvt."""
                with tc.tile_pool(name=f"p1ps{ph}", bufs=2, space="PSUM") as p1ps:
                    for i in range(2):
                        hc = 2 * ph + i
                        pq = p1ps.tile([128, NS], F32, tag="pq", name="pq")
                        for qc in range(QC):
                            nc.tensor.matmul(
                                pq[:, :],
                                wqt[:, qc, i * 128 : (i + 1) * 128],
                                qts[:, qc, :],
                                start=(qc == 0),
                                stop=(qc == QC - 1),
                            )
                        pv = p1ps.tile([128, SL], F32, tag="pv", name="pv")
                        for vc in range(VC):
                            nc.tensor.matmul(
                                pv[:, :],
                                wvt[:, vc, i * 128 : (i + 1) * 128],
                                vts[:, vc, :],
                                start=(vc == 0),
                                stop=(vc == VC - 1),
                            )
                        nc.vector.tensor_scalar_add(
                            QPs[:, hc * NS : (hc + 1) * NS],
                            pq[:, :],
                            wlb[:, HC + hc : HC + hc + 1],
                        )
                        nc.vector.tensor_scalar_add(
                            VP[:, hc * SL : (hc + 1) * SL],
                            pv[:, :],
                            wlb[:, 2 * HC + hc : 2 * HC + hc + 1],
                        )

            def main_hc(hc, b, mp, mid_cb=None):
                """Joint tanh + logit matmuls for one (h-chunk, batch)."""
                P = P0 if b == 0 else P1
                wzb = hc * WZC + (0 if b == 0 else WZ0)
                vcb = hc * SL + b * K0
                qpo = hc * NS + b * NSB
                groups = _groups(hc, b, P)
                for g, js in enumerate(groups):
                    if b == 0 and g == 1 and mid_cb is not None:
                        mid_cb()
                    L = len(js)
                    JT = mp.tile([128, 2 * L * NSB], BF16, tag="JT", name="JT")
                    for kk in range(2 * L):
                        slot = js[kk] if kk < L else js[kk - L] + P
                        nc.vector.tensor_scalar_add(
                            JT[:, kk * NSB : (kk + 1) * NSB],
                            QPs[:, qpo : qpo + NSB],
                            VP[:, vcb + slot : vcb + slot + 1],
                        )
                    # in-place tanh over the whole group
                    nc.scalar.activation(
                        JT[:, :], JT[:, :], mybir.ActivationFunctionType.Tanh
                    )
                    bcs = slice(b * NSB, (b + 1) * NSB)
                    r0 = 64 * b
                    r1 = r0 + 32
                    for jj, j in enumerate(js):
                        first = hc == 0 and g == 0 and jj == 0
                        last = hc == HC - 1 and g == len(groups) - 1 and jj == L - 1
                        nc.tensor.matmul(
                            ps_log[r0 : r0 + P, bcs],
                            wlz[:, wzb + j * P : wzb + (j + 1) * P],
                            JT[:, jj * NSB : (jj + 1) * NSB],
                            start=first,
                            stop=last,
                            tile_position=(0, r0),
                            skip_group_check=True,
                        )
                        nc.tensor.matmul(
                            ps_log[r1 : r1 + P, bcs],
                            wlz[:, wzb + j * P : wzb + (j + 1) * P],
                            JT[:, (L + jj) * NSB : (L + jj + 1) * NSB],
                            start=first,
                            stop=last,
                            tile_position=(0, r1),
                            skip_group_check=True,
                        )

            def proj_b():
                with tc.high_priority():
                    proj_phase(1, wqtB, wvtB)

            proj_phase(0, wqtA, wvtA)
            with tc.tile_pool(name="main", bufs=3) as mp:
                main_hc(0, 0, mp, mid_cb=proj_b)
                main_hc(0, 1, mp)
                for hc in range(1, HC):
                    main_hc(hc, 0, mp)
                    main_hc(hc, 1, mp)

            # ---- masked softmax over slots (no max-pass: |logits| <~ 1.5) ----
            LGA = pp.tile([96 + 32, NS], F32, name="LGA")
            W_all = pp.tile([128, NS // 128, KMAX], F32, name="W_all")
            nc.vector.tensor_copy(LGA[0:P0, 0:NSB], ps_log[0:P0, 0:NSB])
            nc.vector.tensor_copy(
                LGA[32 : 32 + P0, 0:NSB], ps_log[32 : 32 + P0, 0:NSB]
            )
            nc.vector.tensor_copy(
                LGA[64 : 64 + P1, NSB:NS], ps_log[64 : 64 + P1, NSB:NS]
            )
            nc.vector.tensor_copy(
                LGA[96 : 96 + P1, NSB:NS], ps_log[96 : 96 + P1, NSB:NS]
            )
            for nsc in range(NS // 128):
                b = nsc // (NSB // 128)
                P = P0 if b == 0 else P1
                Kb = 2 * P
                r0 = 64 * b
                r1 = r0 + 32
                ps_t = sps.tile([128, KMAX], F32, tag="ps_t", name="ps_t")
                nc.tensor.transpose(
                    ps_t[:, 0:P],
                    LGA[r0 : r0 + P, nsc * 128 : (nsc + 1) * 128],
                    ident[r0 : r0 + P, r0 : r0 + P],
                    tile_position=(r0, 0),
                )
                nc.tensor.transpose(
                    ps_t[:, P : 2 * P],
                    LGA[r1 : r1 + P, nsc * 128 : (nsc + 1) * 128],
                    ident[r1 : r1 + P, r1 : r1 + P],
                    tile_position=(r1, 0),
                )
                LT = pp.tile([128, KMAX], F32, name=f"LT{nsc}")
                nc.vector.tensor_add(
                    LT[:, 0:Kb], ps_t[:, 0:Kb], msk[:, b * K0 : b * K0 + Kb]
                )
                EX = pp.tile([128, KMAX], F32, name=f"EX{nsc}")
                sm = pp.tile([128, 1], F32, name=f"sm{nsc}")
                nc.scalar.activation(
                    EX[:, 0:Kb], LT[:, 0:Kb], mybir.ActivationFunctionType.Exp,
                    accum_out=sm[:, 0:1],
                )
                rs = pp.tile([128, 1], F32, name=f"rs{nsc}")
                nc.vector.reciprocal(rs[:, :], sm[:, :])
                nc.vector.tensor_scalar_mul(
                    W_all[:, nsc, 0:Kb], EX[:, 0:Kb], rs[:, 0:1]
                )
                if Kb < KMAX:
                    nc.vector.memset(W_all[:, nsc, Kb:KMAX], 0.0)
            nc.sync.dma_start(
                out_h[:, :].rearrange("(c p) j -> p c j", p=128), W_all[:, :, :]
            )

    nc.finalize()
    return nc


def _plan(box_mask):
    """Pair batches onto cores large-with-small; return per-core batch ids,
    per-batch unmasked box index lists, and padded slot counts (K0, K1)."""
    mask = np.asarray(box_mask) > 0
    counts = mask.sum(axis=1)
    order = np.argsort(-counts, kind="stable")
    batA = order[:NCORES]
    batB = order[NCORES:][::-1]
    kidx = [np.nonzero(mask[b])[0] for b in range(B)]

    def pad_even(n):
        n = max(int(n), 2)
        return n + (n & 1)

    K0 = pad_even(counts[batA].max())
    K1 = pad_even(counts[batB].max())
    return batA, batB, kidx, K0, K1


def _prep_in_maps(v, q, box_mask, Wv, bv, Wq, bq, Wl, plan):
    """Host-side layout prep: gather unmasked boxes, shard over cores,
    transpose to device layouts."""
    import ml_dtypes

    batA, batB, kidx, K0, K1 = plan
    P0, P1 = K0 // 2, K1 // 2
    SL = K0 + K1
    WZ0 = P0 * P0
    WZC = WZ0 + P1 * P1

    v = np.asarray(v, np.float32).reshape(B, K, VD)
    q = np.asarray(q, np.float32).reshape(B, N * S, QD)

    WqT = np.asarray(Wq, np.float32).T                                # [QD, H]
    WvT = np.asarray(Wv, np.float32).T                                # [VD, H]
    WqTA = np.ascontiguousarray(WqT[:, :256]).astype(ml_dtypes.bfloat16)
    WqTB = np.ascontiguousarray(WqT[:, 256:]).astype(ml_dtypes.bfloat16)
    WvTA = np.ascontiguousarray(WvT[:, :256]).astype(ml_dtypes.bfloat16)
    WvTB = np.ascontiguousarray(WvT[:, 256:]).astype(ml_dtypes.bfloat16)
    wlb = np.zeros((128, 12), np.float32)
    wl_chunks = np.asarray(Wl, np.float32).reshape(HC, 128).T         # [128, hc]
    wlb[:, 0:4] = wl_chunks
    wlb[:, 4:8] = np.asarray(bq, np.float32).reshape(HC, 128).T
    wlb[:, 8:12] = np.asarray(bv, np.float32).reshape(HC, 128).T
    # zero-padded Wl variants per (hc, b, j)
    wlz = np.zeros((128, HC, WZC), np.float32)
    for j in range(P0):
        wlz[:, :, j * P0 + j] = wl_chunks
    for j in range(P1):
        wlz[:, :, WZ0 + j * P1 + j] = wl_chunks
    wlz = wlz.reshape(128, HC * WZC).astype(ml_dtypes.bfloat16)
    ident = np.eye(128, dtype=np.float32)

    in_maps = []
    for c in range(NCORES):
        qc = np.concatenate(
            [q[batA[c]], q[batB[c]]], axis=0
        )                                                             # [NS, QD]
        qT = np.ascontiguousarray(qc.T).astype(ml_dtypes.bfloat16)    # [QD, NS]
        vg = np.zeros((SL, VD), np.float32)
        moff = np.full((1, SL), -1e9, np.float32)
        for b, (bi, Kb, off) in enumerate(
            [(batA[c], K0, 0), (batB[c], K1, K0)]
        ):
            ks = kidx[bi]
            vg[off : off + len(ks)] = v[bi, ks]
            moff[0, off : off + len(ks)] = 0.0
        vG = np.ascontiguousarray(vg.T).astype(ml_dtypes.bfloat16)    # [VD, SL]
        msk = np.ascontiguousarray(np.broadcast_to(moff, (128, SL)))
        in_maps.append(
            {
                "qT": qT,
                "vG": vG,
                "WqTA": WqTA,
                "WqTB": WqTB,
                "WvTA": WvTA,
                "WvTB": WvTB,
                "wlb": wlb,
                "wlz": wlz,
                "msk": msk,
                "ident": ident,
            }
        )
    return in_maps


def kernel(v, q, box_mask, tags_attention, Wv, bv, Wq, bq, Wl, bl):
    # bl shifts all unmasked logits uniformly -> cancels in softmax.
    # tags_attention is unused by the reference module.
    plan = _plan(box_mask)
    batA, batB, kidx, K0, K1 = plan
    key = (K0, K1)
    if key not in _CACHE:
        _CACHE[key] = _build_nc(K0, K1)
    nc = _CACHE[key]
    in_maps = _prep_in_maps(v, q, box_mask, Wv, bv, Wq, bq, Wl, plan)
    res = bass_utils.run_bass_kernel_spmd(
        nc,
        in_maps,
        core_ids=list(range(NCORES)),
        trace=bool(os.environ.get("KERNEL_TRACE")),
        tmpdir=os.environ.get("KERNEL_TMPDIR"),
    )
    _CACHE["last_result"] = res
    out = np.zeros((B, N * S, K), np.float32)
    for c in range(NCORES):
        w = res.results[c]["out"]                                     # [NS, KMAX]
        for b, (bi, off) in enumerate([(batA[c], 0), (batB[c], NSB)]):
            ks = kidx[bi]
            if len(ks) == 0:
                out[bi, :, :] = 1.0 / K
            else:
                out[bi, :, ks] = w[off : off + NSB, : len(ks)].T
    return out.reshape(B, N, S, K)


# revision 9
# speedup vs baseline: 1.5414x; 1.0352x over previous
"""Fused additive-attention kernel for Trainium2 (8 NeuronCores, SPMD).

Computes  w = softmax_K( mask ? (Wl . tanh(vW_v^T + qW_q^T) + bl) : -1e9 )
without materializing the [B,N,S,K,H] joint_repr intermediate.

Key ideas over the naive formulation:
  * Masked boxes get weight exactly 0 (exp(-1e9) underflows), so only the
    unmasked boxes are computed.  Host gathers each batch's unmasked box
    list; batches are paired onto cores large-with-small so the padded
    per-slot counts (K0 for the core's first batch, K1 for its second) stay
    near the true max.  Masked/padding slots are -1e9'd on device and the
    host scatters results back into the full [B,N,S,K] output (zeros for
    masked boxes).
  * The broadcast add vp[b,k,h] + qp[b,n,s,h] runs as DVE tensor_scalar_add
    with a per-partition [128,1] vp operand (high DVE perf mode).
  * All device tensors are packed on host into their SBUF layout
    ([128, ...] partition-major, contiguous per partition) so every DMA is
    large-descriptor and the single queue is bandwidth- not
    descriptor-rate-bound.  Weights are split per h-chunk so compute starts
    after ~1/3 of the bytes.
  * Biases bq/bv are folded into QPs/VP at projection time; bl cancels in
    softmax.  Logits are bounded (|logit| <= sum|Wl|), so softmax skips the
    max-subtraction pass.

Per-core dataflow (h on partitions for the hot loop), phased per h-chunk:
  QPs[hc] [128(h), 512(b,ns)] = Wq-slice.T @ qT + bq   (PE psum, DVE copy)
  VP[hc]  [128(h), S(slots)]  = Wv-slice.T @ vG + bv   (S = K0+K1)
  JT      [128, strip(kk)*256] bf16 = QPs-half + VP[slot]  (DVE)
  tanh in-place on JT (one ACT op per slot-group)
  logits  psum: batch b uses cols b*256:(b+1)*256 and PE col-strips
          (0, 32) for b0 / (64, 96) for b1 (disjoint psum partition rows:
          start=True zeroes the whole 2KB bank row), accumulated over hc
          with zero-padded Wl lhsT (pair j, j+P shares lhsT; tile_position).
  softmax over slots after PE-transposing logits to [ns, slots];
  hc3 runs b1 before b0 so b1's softmax hides under b0's tanh stream.
"""

import os
import sys

import numpy as np

sys.path.insert(0, "/opt/trn_rl_repo")

import concourse.bass as bass
import concourse.mybir as mybir
from concourse import bacc, bass_utils
from concourse.tile import TileContext

# Problem shapes (hardcoded per contract -- kernel.py must be self-contained)
B, N, S, K = 16, 4, 64, 50
VD, QD, H = 1024, 768, 512
NCORES = 8
BPC = B // NCORES          # batches per core = 2
NS = BPC * N * S           # 512 rows (b, n, s) per core
NSB = NS // BPC            # 256 rows per batch
HC = H // 128              # 4 h-chunks
QC = QD // 128             # 6 qd-chunks
VC = VD // 128             # 8 vd-chunks

F32 = mybir.dt.float32
BF16 = mybir.dt.bfloat16

_CACHE = {}


def _groups(hc, b, P):
    """Pair-index groups for (hc, batch).  First group of the whole kernel is
    tiny so the first tanh issues as early as possible."""
    pairs = list(range(P))
    if hc == 0 and b == 0:
        gs = [pairs[0:2], pairs[2 : P // 2 + 1], pairs[P // 2 + 1 :]]
    else:
        h = (P + 1) // 2
        gs = [pairs[:h], pairs[h:]]
    return [g for g in gs if g]


def _build_nc(K0, K1):
    P0, P1 = K0 // 2, K1 // 2
    SL = K0 + K1               # slots per core
    KMAX = max(K0, K1)
    WZ0 = P0 * P0              # wlz cols per hc for batch 0
    WZC = WZ0 + P1 * P1        # wlz cols per hc total

    nc = bacc.Bacc("TRN2", target_bir_lowering=False)

    # All inputs are pre-packed on host into SBUF layout [128, ...]
    qT_h = nc.dram_tensor("qT", [128, QC * NS], BF16, kind="ExternalInput")
    vG_h = nc.dram_tensor("vG", [128, VC * SL], BF16, kind="ExternalInput")
    wq_h = [
        nc.dram_tensor(f"wq{hc}", [128, QC * 128], BF16, kind="ExternalInput")
        for hc in range(HC)
    ]
    wv_h = [
        nc.dram_tensor(f"wv{hc}", [128, VC * 128], BF16, kind="ExternalInput")
        for hc in range(HC)
    ]
    # packed [128, 12]: cols 0:4 Wl chunks, 4:8 bq chunks, 8:12 bv chunks
    wlb_h = nc.dram_tensor("wlb", [128, 12], F32, kind="ExternalInput")
    # zero-padded Wl variants, per (hc, b, j): [128, Pb] slab, col c = Wl*(c==j)
    wlz_h = nc.dram_tensor("wlz", [128, HC * WZC], BF16, kind="ExternalInput")
    # additive mask: col s = 0.0 for a real slot, -1e9 for padding/masked
    msk_h = nc.dram_tensor("msk", [128, SL], F32, kind="ExternalInput")
    id_h = nc.dram_tensor("ident", [128, 128], F32, kind="ExternalInput")
    # out col (nsc, j): w[ns = nsc*128 + p, slot j]
    out_h = nc.dram_tensor(
        "out", [128, (NS // 128) * KMAX], F32, kind="ExternalOutput"
    )

    with TileContext(nc) as tc:
        with (
            tc.tile_pool(name="persist", bufs=1) as pp,
            tc.tile_pool(name="ppsum", bufs=1, space="PSUM") as ppsum,
            tc.tile_pool(name="projps", bufs=2, space="PSUM") as pjps,
            tc.tile_pool(name="smpsum", bufs=2, space="PSUM") as sps,
        ):
            # ---- DMA loads, ordered so the hc0 projection chain starts
            # after ~1/3 of the total input bytes ----
            wlb = pp.tile_from(wlb_h[:, :], name="wlb")
            qts = pp.tile([128, QC, NS], BF16, name="qts")
            qts_f = qts[:, :, :].rearrange("p c j -> p (c j)")
            nc.sync.dma_start(qts_f[:, 0 : 2 * NS], qT_h[:, 0 : 2 * NS])
            wqt = [None] * HC
            wvt = [None] * HC
            wqt[0] = pp.tile_from(wq_h[0][:, :], name="wq0")
            nc.sync.dma_start(qts_f[:, 2 * NS : 4 * NS], qT_h[:, 2 * NS : 4 * NS])
            nc.sync.dma_start(qts_f[:, 4 * NS : 6 * NS], qT_h[:, 4 * NS : 6 * NS])
            vts = pp.tile_from(vG_h[:, :], name="vts")
            wvt[0] = pp.tile_from(wv_h[0][:, :], name="wv0")
            wqt[1] = pp.tile_from(wq_h[1][:, :], name="wq1")
            wvt[1] = pp.tile_from(wv_h[1][:, :], name="wv1")
            wlz = pp.tile_from(wlz_h[:, :], name="wlz")
            msk = pp.tile_from(msk_h[:, :], name="msk")
            ident = pp.tile_from(id_h[:, :], name="ident")
            wqt[2] = pp.tile_from(wq_h[2][:, :], name="wq2")
            wvt[2] = pp.tile_from(wv_h[2][:, :], name="wv2")
            wqt[3] = pp.tile_from(wq_h[3][:, :], name="wq3")
            wvt[3] = pp.tile_from(wv_h[3][:, :], name="wv3")

            # qp (all h-chunks): [128, (hc, b, ns)] bf16, +bq folded
            QPs = pp.tile([128, HC * NS], BF16, name="QPs")
            # vp slot table: [128, (hc, slot)] f32, +bv folded
            VP = pp.tile([128, HC * SL], F32, name="VP")

            # logits psum: batch b owns cols b*256:(b+1)*256 and PE col-strips
            # (0, 32) for b0 / (64, 96) for b1 -> psum rows 0:P0, 32:32+P0,
            # 64:64+P1, 96:96+P1.  Strips of the two batches must not share
            # psum partition rows: start=True zeroes the whole 2KB bank row.
            ps_log = ppsum.tile([128, NS], F32, name="ps_log")

            def proj_phase(hc):
                """Compute QPs/VP h-chunk hc."""
                pq = pjps.tile([128, NS], F32, tag="pq", name="pq")
                for qc in range(QC):
                    nc.tensor.matmul(
                        pq[:, :],
                        wqt[hc][:, qc * 128 : (qc + 1) * 128],
                        qts[:, qc, :],
                        start=(qc == 0),
                        stop=(qc == QC - 1),
                    )
                pv = pjps.tile([128, SL], F32, tag="pv", name="pv")
                for vc in range(VC):
                    nc.tensor.matmul(
                        pv[:, :],
                        wvt[hc][:, vc * 128 : (vc + 1) * 128],
                        vts[:, vc * SL : (vc + 1) * SL],
                        start=(vc == 0),
                        stop=(vc == VC - 1),
                    )
                # split the hc0 copies so the first adds start sooner
                nsplit = 2 if hc == 0 else 1
                for i in range(nsplit):
                    sl = slice(i * NS // nsplit, (i + 1) * NS // nsplit)
                    nc.vector.tensor_scalar_add(
                        QPs[:, hc * NS :][:, sl],
                        pq[:, sl],
                        wlb[:, HC + hc : HC + hc + 1],
                    )
                nc.vector.tensor_scalar_add(
                    VP[:, hc * SL : (hc + 1) * SL],
                    pv[:, :],
                    wlb[:, 2 * HC + hc : 2 * HC + hc + 1],
                )

            def main_hc(hc, b, mp, mid_cb=None):
                """Joint tanh + logit matmuls for one (h-chunk, batch)."""
                P = P0 if b == 0 else P1
                wzb = hc * WZC + (0 if b == 0 else WZ0)
                vcb = hc * SL + b * K0
                qpo = hc * NS + b * NSB
                groups = _groups(hc, b, P)
                for g, js in enumerate(groups):
                    if g == 1 and mid_cb is not None:
                        mid_cb()
                    L = len(js)
                    JT = mp.tile([128, 2 * L * NSB], BF16, tag="JT", name="JT")
                    for kk in range(2 * L):
                        slot = js[kk] if kk < L else js[kk - L] + P
                        nc.vector.tensor_scalar_add(
                            JT[:, kk * NSB : (kk + 1) * NSB],
                            QPs[:, qpo : qpo + NSB],
                            VP[:, vcb + slot : vcb + slot + 1],
                        )
                    # in-place tanh over the whole group
                    nc.scalar.activation(
                        JT[:, :], JT[:, :], mybir.ActivationFunctionType.Tanh
                    )
                    bcs = slice(b * NSB, (b + 1) * NSB)
                    r0 = 64 * b
                    r1 = r0 + 32
                    for jj, j in enumerate(js):
                        first = hc == 0 and g == 0 and jj == 0
                        last = hc == HC - 1 and g == len(groups) - 1 and jj == L - 1
                        nc.tensor.matmul(
                            ps_log[r0 : r0 + P, bcs],
                            wlz[:, wzb + j * P : wzb + (j + 1) * P],
                            JT[:, jj * NSB : (jj + 1) * NSB],
                            start=first,
                            stop=last,
                            tile_position=(0, r0),
                            skip_group_check=True,
                        )
                        nc.tensor.matmul(
                            ps_log[r1 : r1 + P, bcs],
                            wlz[:, wzb + j * P : wzb + (j + 1) * P],
                            JT[:, (L + jj) * NSB : (L + jj + 1) * NSB],
                            start=first,
                            stop=last,
                            tile_position=(0, r1),
                            skip_group_check=True,
                        )

            LGA = pp.tile([96 + 32, NSB], F32, name="LGA")
            W_all = pp.tile([128, NS // 128, KMAX], F32, name="W_all")

            def softmax_b(b):
                """Masked softmax for batch b (no max-pass: |logits| <~ 1.5)."""
                P = P0 if b == 0 else P1
                Kb = 2 * P
                r0 = 64 * b
                r1 = r0 + 32
                bcs = slice(b * NSB, (b + 1) * NSB)
                nc.vector.tensor_copy(LGA[r0 : r0 + P, :], ps_log[r0 : r0 + P, bcs])
                nc.vector.tensor_copy(LGA[r1 : r1 + P, :], ps_log[r1 : r1 + P, bcs])
                for nsb in range(NSB // 128):
                    nsc = b * 2 + nsb
                    ps_t = sps.tile([128, KMAX], F32, tag="ps_t", name="ps_t")
                    nc.tensor.transpose(
                        ps_t[:, 0:P],
                        LGA[r0 : r0 + P, nsb * 128 : (nsb + 1) * 128],
                        ident[r0 : r0 + P, r0 : r0 + P],
                        tile_position=(r0, 0),
                    )
                    nc.tensor.transpose(
                        ps_t[:, P : 2 * P],
                        LGA[r1 : r1 + P, nsb * 128 : (nsb + 1) * 128],
                        ident[r1 : r1 + P, r1 : r1 + P],
                        tile_position=(r1, 0),
                    )
                    LT = pp.tile([128, KMAX], F32, name=f"LT{nsc}")
                    nc.vector.tensor_add(
                        LT[:, 0:Kb], ps_t[:, 0:Kb], msk[:, b * K0 : b * K0 + Kb]
                    )
                    EX = pp.tile([128, KMAX], F32, name=f"EX{nsc}")
                    sm = pp.tile([128, 1], F32, name=f"sm{nsc}")
                    nc.scalar.activation(
                        EX[:, 0:Kb], LT[:, 0:Kb],
                        mybir.ActivationFunctionType.Exp,
                        accum_out=sm[:, 0:1],
                    )
                    rs = pp.tile([128, 1], F32, name=f"rs{nsc}")
                    nc.vector.reciprocal(rs[:, :], sm[:, :])
                    nc.vector.tensor_scalar_mul(
                        W_all[:, nsc, 0:Kb], EX[:, 0:Kb], rs[:, 0:1]
                    )
                    if Kb < KMAX:
                        nc.vector.memset(W_all[:, nsc, Kb:KMAX], 0.0)
                out_v = out_h[:, :].rearrange("p (c j) -> p c j", j=KMAX)
                nc.sync.dma_start(
                    out_v[:, 2 * b : 2 * b + 2, :], W_all[:, 2 * b : 2 * b + 2, :]
                )

            proj_phase(0)
            with tc.tile_pool(name="main", bufs=4) as mp:

                def prio(hc):
                    def cb():
                        with tc.high_priority():
                            proj_phase(hc)

                    return cb

                main_hc(0, 0, mp, mid_cb=prio(1))
                main_hc(0, 1, mp, mid_cb=prio(2))
                main_hc(1, 0, mp, mid_cb=prio(3))
                main_hc(1, 1, mp)
                main_hc(2, 0, mp)
                main_hc(2, 1, mp)
                main_hc(3, 1, mp)   # b1 first: its softmax hides under b0 tanh
                main_hc(3, 0, mp)
                softmax_b(1)
                softmax_b(0)

    nc.finalize()
    return nc


def _plan(box_mask):
    """Pair batches onto cores large-with-small; return per-core batch ids,
    per-batch unmasked box index lists, and padded slot counts (K0, K1)."""
    mask = np.asarray(box_mask) > 0
    counts = mask.sum(axis=1)
    order = np.argsort(-counts, kind="stable")
    batA = order[:NCORES]
    batB = order[NCORES:][::-1]
    kidx = [np.nonzero(mask[b])[0] for b in range(B)]

    def pad_even(n):
        n = max(int(n), 2)
        return n + (n & 1)

    K0 = pad_even(counts[batA].max())
    K1 = pad_even(counts[batB].max())
    return batA, batB, kidx, K0, K1


def _dev_pack(M):
    """[C*128, X] row-major -> [128, C*X] partition-major (SBUF layout)."""
    D, X = M.shape
    C = D // 128
    return np.ascontiguousarray(
        M.reshape(C, 128, X).transpose(1, 0, 2).reshape(128, C * X)
    )


def _prep_in_maps(v, q, box_mask, Wv, bv, Wq, bq, Wl, plan):
    """Host-side layout prep: gather unmasked boxes, shard over cores,
    pack into device layouts."""
    import ml_dtypes

    batA, batB, kidx, K0, K1 = plan
    P0, P1 = K0 // 2, K1 // 2
    SL = K0 + K1
    WZ0 = P0 * P0
    WZC = WZ0 + P1 * P1

    v = np.asarray(v, np.float32).reshape(B, K, VD)
    q = np.asarray(q, np.float32).reshape(B, N * S, QD)

    WqT = np.asarray(Wq, np.float32).T                                # [QD, H]
    WvT = np.asarray(Wv, np.float32).T                                # [VD, H]
    shared = {}
    for hc in range(HC):
        shared[f"wq{hc}"] = _dev_pack(
            np.ascontiguousarray(WqT[:, hc * 128 : (hc + 1) * 128])
        ).astype(ml_dtypes.bfloat16)
        shared[f"wv{hc}"] = _dev_pack(
            np.ascontiguousarray(WvT[:, hc * 128 : (hc + 1) * 128])
        ).astype(ml_dtypes.bfloat16)
    wlb = np.zeros((128, 12), np.float32)
    wl_chunks = np.asarray(Wl, np.float32).reshape(HC, 128).T         # [128, hc]
    wlb[:, 0:4] = wl_chunks
    wlb[:, 4:8] = np.asarray(bq, np.float32).reshape(HC, 128).T
    wlb[:, 8:12] = np.asarray(bv, np.float32).reshape(HC, 128).T
    shared["wlb"] = wlb
    # zero-padded Wl variants per (hc, b, j)
    wlz = np.zeros((128, HC, WZC), np.float32)
    for j in range(P0):
        wlz[:, :, j * P0 + j] = wl_chunks
    for j in range(P1):
        wlz[:, :, WZ0 + j * P1 + j] = wl_chunks
    shared["wlz"] = wlz.reshape(128, HC * WZC).astype(ml_dtypes.bfloat16)
    shared["ident"] = np.eye(128, dtype=np.float32)

    in_maps = []
    for c in range(NCORES):
        qc = np.concatenate(
            [q[batA[c]], q[batB[c]]], axis=0
        )                                                             # [NS, QD]
        qT = np.ascontiguousarray(qc.T)                               # [QD, NS]
        vg = np.zeros((SL, VD), np.float32)
        moff = np.full((1, SL), -1e9, np.float32)
        for bi, Kb, off in [(batA[c], K0, 0), (batB[c], K1, K0)]:
            ks = kidx[bi]
            vg[off : off + len(ks)] = v[bi, ks]
            moff[0, off : off + len(ks)] = 0.0
        vG = np.ascontiguousarray(vg.T)                               # [VD, SL]
        msk = np.ascontiguousarray(np.broadcast_to(moff, (128, SL)))
        in_maps.append(
            {
                "qT": _dev_pack(qT).astype(ml_dtypes.bfloat16),
                "vG": _dev_pack(vG).astype(ml_dtypes.bfloat16),
                "msk": msk,
                **shared,
            }
        )
    return in_maps


def kernel(v, q, box_mask, tags_attention, Wv, bv, Wq, bq, Wl, bl):
    # bl shifts all unmasked logits uniformly -> cancels in softmax.
    # tags_attention is unused by the reference module.
    plan = _plan(box_mask)
    batA, batB, kidx, K0, K1 = plan
    KMAX = max(K0, K1)
    key = (K0, K1)
    if key not in _CACHE:
        _CACHE[key] = _build_nc(K0, K1)
    nc = _CACHE[key]
    in_maps = _prep_in_maps(v, q, box_mask, Wv, bv, Wq, bq, Wl, plan)
    res = bass_utils.run_bass_kernel_spmd(
        nc,
        in_maps,
        core_ids=list(range(NCORES)),
        trace=bool(os.environ.get("KERNEL_TRACE")),
        tmpdir=os.environ.get("KERNEL_TMPDIR"),
    )
    _CACHE["last_result"] = res
    out = np.zeros((B, N * S, K), np.float32)
    for c in range(NCORES):
        w = (
            res.results[c]["out"]
            .reshape(128, NS // 128, KMAX)
            .transpose(1, 0, 2)
            .reshape(NS, KMAX)
        )
        for bi, off in [(batA[c], 0), (batB[c], NSB)]:
            ks = kidx[bi]
            if len(ks) == 0:
                out[bi, :, :] = 1.0 / K
            else:
                out[bi, :, ks] = w[off : off + NSB, : len(ks)].T
    return out.reshape(B, N, S, K)
